# revision 43
# baseline (speedup 1.0000x reference)
"""Distributed real SHT (spherical harmonic transform) on 8 trn2 NeuronCores.

Pipeline:
  out[b,c,l,m] = sum_k W[m,l,k] * XF[b,c,m,k],   XF = (2*pi/nlon) * rfft(x, lon)[..., :mmax]

Stage A (launch 1, channel-sharded): DFT along longitude as bf16 matmuls.
  Host folds x over lon parity (cos: n'=0..360, sin: n'=1..359) and packs
  GROUPS of 4 channels per DMA so every transfer is >=0.6 MB with >=2.9 KB
  contiguous per-partition runs (descriptor-efficient; single-queue BW was
  measured 112 GB/s at 0.7 KB runs vs 200 GB/s at 4.3 KB).
  psum[k_tile, m] += xT[n'chunk, k_tile]^T @ DFTmat[n'chunk, m]
Host exchange: XF[c,k,m] (channel-sharded) -> per-core m-sharded, p-major
  chunk layout (partition-major so per-partition runs are nkc*1KB).
Stage B (launch 2, m-sharded): psum[l_tile, 512] += WT[k, l_tile]^T @ XFB[k, 512]
  Only the latitude window where P_l^m is non-negligible is loaded and
  contracted, with EXACT spans (partial last chunk via a separate DMA +
  partial-partition matmul) instead of 128-row padding.

All DMA traffic is spread across the three queues (sync/scalar HWDGE +
gpsimd SWDGE) roughly by their measured throughputs. bf16 operands keep
the PE at 2.4 GHz and halve DMA bytes; psum accumulation is fp32.
"""

import os

import numpy as np

import concourse.bacc as bacc
import concourse.mybir as mybir
from concourse.tile import TileContext
from concourse.bass_utils import run_bass_kernel_spmd

LAST_PERF = {}

NLAT = 361
NLON = 720
MMAX = 361
LMAX = 361
C = 256
NCORES = 8
CPC = C // NCORES  # 32 channels per core
NC_COS = NLON // 2 + 1  # 361 cos columns (n'=0..360)
NC_SIN = NLON // 2 - 1  # 359 sin columns (n'=1..359)
MPC = (MMAX + NCORES - 1) // NCORES  # 46 m's per core (padded)
MEVEN = 362  # m padded even (stage A moving free dim)
GA = 4  # channels per stage-A DMA group
NGA = CPC // GA  # 8 groups per core
NKC_MAX = 3  # max 128-row latitude chunks in stage B
NRIC = 2 * C  # 512 = (re|im) x 256 channels

F32 = mybir.dt.float32
BF16 = mybir.dt.bfloat16

K_TILES = [(0, 128), (128, 128), (256, 105)]


def _ptiles(n, p=128):
    out = []
    o = 0
    while o < n:
        out.append((o, min(p, n - o)))
        o += p
    return out


def build_stage_a(mts):
    """xin [NGA, 768, GA*362] bf16: row r = DFT contraction row (cos rows
    0:361 in segs 0-2, sin rows 384:743 in segs 3-5), col = c*362 + j where
    j indexes PERMUTED latitudes (mirror-paired k-tiles: pole rows first).
    mats [768, 362] bf16 (same row packing, cols = m, col 361 zero).
    xf{t} [NGA, kp_t, GA*2*mts[t]]: row = permuted latitude within tile t,
    col = c*(2*mt) + ri*mt + m.  mts[t] = m-columns kept for tile t (the
    pole tiles only need small m: P_l^m is negligible there)."""
    nc = bacc.Bacc("TRN2", target_bir_lowering=False)
    xin = nc.dram_tensor("xin", [NGA, 768, GA * MEVEN], BF16, kind="ExternalInput")
    mats = nc.dram_tensor("mats", [768, MEVEN], BF16, kind="ExternalInput")
    xfs = [
        nc.dram_tensor(f"xf{t}", [NGA, kp, GA * 2 * mts[t]], BF16, kind="ExternalOutput")
        for t, (k0, kp) in enumerate(K_TILES)
    ]

    with TileContext(nc) as tc:
        with (
            tc.tile_pool(name="mats", bufs=1) as matp,
            tc.tile_pool(name="xinp", bufs=4) as xinp,
            tc.tile_pool(name="outp", bufs=8) as outp,
            tc.tile_pool(name="ps", bufs=8, space="PSUM") as psp,
        ):
            mat_t = matp.tile([128, 6 * MEVEN], BF16, tag="mats")
            # cos half first on sync (first matmuls need it), sin on scalar
            nc.sync.dma_start(
                out=mat_t[:, : 3 * MEVEN].rearrange("p (s m) -> p s m", s=3),
                in_=mats[:384].rearrange("(s p) m -> p s m", p=128),
            )
            nc.scalar.dma_start(
                out=mat_t[:, 3 * MEVEN :].rearrange("p (s m) -> p s m", s=3),
                in_=mats[384:].rearrange("(s p) m -> p s m", p=128),
            )
            copy_i = 0
            pending = []
            for g in range(NGA):
                x_t = xinp.tile([128, 6 * GA * MEVEN], BF16, tag="xin")
                if g == 0:
                    # fine-grained ramp: first matmul only needs the cos
                    # matrix + channels 0-1 of the cos half (~0.56 MB)
                    half = 3 * GA * MEVEN
                    for cc in range(2):
                        nc.sync.dma_start(
                            out=x_t[:, :half].rearrange(
                                "p (s c k) -> p s c k", s=3, c=GA
                            )[:, :, 2 * cc : 2 * cc + 2, :],
                            in_=xin[g, :384].rearrange(
                                "(s p) (c k) -> p s c k", p=128, c=GA
                            )[:, :, 2 * cc : 2 * cc + 2, :],
                        )
                        nc.scalar.dma_start(
                            out=x_t[:, half:].rearrange(
                                "p (s c k) -> p s c k", s=3, c=GA
                            )[:, :, 2 * cc : 2 * cc + 2, :],
                            in_=xin[g, 384:].rearrange(
                                "(s p) (c k) -> p s c k", p=128, c=GA
                            )[:, :, 2 * cc : 2 * cc + 2, :],
                        )
                else:
                    # gpsimd is too slow (~120 GB/s) for critical-path loads
                    (nc.scalar if g % 2 else nc.sync).dma_start(
                        out=x_t.rearrange("p (s f) -> p s f", s=6),
                        in_=xin[g].rearrange("(s p) f -> p s f", p=128),
                    )
                # previous group's stores go here: behind this load in FIFO
                if pending:
                    pending.pop(0)()
                ots = [
                    outp.tile([128, GA * 2 * mts[kt]], BF16, tag=f"ot{kt}", name=f"ot{kt}")
                    for kt in range(len(K_TILES))
                ]
                for ri in range(2):
                    for c in range(GA):
                        for kt, (k0, kp) in enumerate(K_TILES):
                            mt = mts[kt]
                            ps = psp.tile([128, MEVEN], F32, tag="ps")
                            for s in range(3):
                                seg = 3 * ri + s
                                base = (seg * GA + c) * MEVEN
                                nc.tensor.matmul(
                                    ps[:kp, :mt],
                                    x_t[:, base + k0 : base + k0 + kp],
                                    mat_t[:, seg * MEVEN : seg * MEVEN + mt],
                                    start=(s == 0),
                                    stop=(s == 2),
                                )
                            dst = ots[kt][:kp, (c * 2 + ri) * mt : (c * 2 + ri + 1) * mt]
                            if copy_i % 3 != 2:  # 2/3 DVE, 1/3 ACT (ACT ~2x slower)
                                nc.vector.tensor_copy(out=dst, in_=ps[:kp, :mt])
                            else:
                                nc.scalar.copy(dst, ps[:kp, :mt])
                            copy_i += 1
                def _stores(g=g, ots=ots):
                    for kt, (k0, kp) in enumerate(K_TILES):
                        if g >= NGA - 2:  # tail: drain the last groups 3-wide
                            st = (nc.gpsimd, nc.sync, nc.scalar)[kt]
                        elif kt < 2:  # gpsimd: stores only, off the load path
                            st = nc.gpsimd
                        else:
                            st = nc.sync if g % 2 == 0 else nc.scalar
                        st.dma_start(out=xfs[kt][g], in_=ots[kt][:kp, :])
                pending.append(_stores)
            for fn in pending:
                fn()
    nc.compile()
    return nc


XW_W = 5238  # max per-pair flat width: 2*nkc*(512+Lp) at pair 0


def build_stage_b(nkc_list, rc_list):
    """Index i handles m = 8*i + core_j; computes l in [8*i, lmax).
    All inputs for an index PAIR are packed into one flat row-contiguous
    block so each pair is ONE clean 2D DMA (2-10 KB runs, exact bytes):
      xw [23, 128, XW_W]: row p (= latitude chunk row) holds
        [i0 xfb (nkc*512) | i1 xfb | i0 wt (nkc*Lp) | i1 wt]
      where xfb col t*512+f = XF at latitude (klo + t*rc + p), f = ri*256+c,
      and wt col t*Lp+lc = W[m, same latitude, 8*i + lc] (odd i: top 8 cols 0).
    outb [23, 128, 3072]: cols [i0 full l-tiles | i1 fulls | i0 part | i1 part].
    nkc/rc uniform within each pair; rows beyond each window zero-filled."""
    nc = bacc.Bacc("TRN2", target_bir_lowering=False)
    xw = nc.dram_tensor("xw", [MPC // 2, 128, XW_W], BF16, kind="ExternalInput")
    outb = nc.dram_tensor("outb", [MPC // 2, 128, 6 * NRIC], BF16, kind="ExternalOutput")

    with TileContext(nc) as tc:
        with (
            tc.tile_pool(name="xwp", bufs=12) as xwp,
            tc.tile_pool(name="outp", bufs=8) as outp,
            tc.tile_pool(name="ps", bufs=8, space="PSUM") as psp,
        ):
            copy_i = 0
            pending = []
            for pp in range(MPC // 2):
                pi = 2 * pp
                nkc = nkc_list[pi]
                rc = rc_list[pi]
                Lp = LMAX - 8 * pi  # wt col width shared within the pair
                used = 2 * nkc * (NRIC + Lp)
                w0 = 2 * nkc * NRIC  # wt region base
                xw_t = xwp.tile([128, XW_W], BF16, tag="xw")
                ot = outp.tile([128, 6 * NRIC], BF16, tag="ot")
                eng = nc.sync if pp % 2 == 0 else nc.scalar
                if pp < 2:
                    # split across queues so the first matmul starts sooner
                    nc.sync.dma_start(
                        out=xw_t[:, : nkc * NRIC], in_=xw[pp, :, : nkc * NRIC]
                    )
                    nc.scalar.dma_start(
                        out=xw_t[:, w0 : w0 + nkc * Lp],
                        in_=xw[pp, :, w0 : w0 + nkc * Lp],
                    )
                    nc.sync.dma_start(
                        out=xw_t[:, nkc * NRIC : w0], in_=xw[pp, :, nkc * NRIC : w0]
                    )
                    nc.scalar.dma_start(
                        out=xw_t[:, w0 + nkc * Lp : used],
                        in_=xw[pp, :, w0 + nkc * Lp : used],
                    )
                else:
                    eng.dma_start(out=xw_t[:, :used], in_=xw[pp, :, :used])
                nfull = len(_ptiles(Lp)) - 1  # same within the pair
                for il in range(2):
                    Li = LMAX - 8 * (pi + il)
                    for tl, (l0, lp) in enumerate(_ptiles(Li)):
                        ps = psp.tile([128, NRIC], F32, tag="ps")
                        for t in range(nkc):
                            nc.tensor.matmul(
                                ps[:lp, :],
                                xw_t[:, w0 + il * nkc * Lp + t * Lp + l0 : w0 + il * nkc * Lp + t * Lp + l0 + lp],
                                xw_t[:, il * nkc * NRIC + t * NRIC : il * nkc * NRIC + (t + 1) * NRIC],
                                start=(t == 0),
                                stop=(t == nkc - 1),
                            )
                        if tl < nfull:
                            dst = ot[:lp, (il * nfull + tl) * NRIC : (il * nfull + tl + 1) * NRIC]
                        else:
                            dst = ot[:lp, (2 * nfull + il) * NRIC : (2 * nfull + il + 1) * NRIC]
                        # all copies on DVE: it issues no DMAs, so its FIFO
                        # never blocks behind a waiting dma_start (ACT's does)
                        nc.vector.tensor_copy(out=dst, in_=ps[:lp, :])
                        copy_i += 1
                # ONE full-height store per pair (partial-tile garbage rows
                # are ignored by the host), emitted 2 iterations late so it
                # sits BEHIND the next pairs' loads in the engine FIFOs.
                def _store(pp=pp, nfull=nfull, ot=ot):
                    st = (nc.gpsimd, nc.sync, nc.scalar)[pp % 3]
                    st.dma_start(
                        out=outb[pp, :, : (2 * nfull + 2) * NRIC],
                        in_=ot[:, : (2 * nfull + 2) * NRIC],
                    )
                pending.append(_store)
                if len(pending) > 2:
                    pending.pop(0)()
            for fn in pending:
                fn()
    nc.compile()
    return nc


def _dft_matrices():
    """cosm[n', m] = s*cos(2 pi m n'/nlon), n'=0..360
    sinm[n', m] = -s*sin(2 pi m n'/nlon), n'=1..359 (imag of rfft = -sum x sin)."""
    s = 2.0 * np.pi / NLON
    m = np.arange(MMAX)
    nc_ = np.arange(NC_COS)
    ns_ = np.arange(1, NLON // 2)
    ang_c = 2.0 * np.pi * ((nc_[:, None] * m[None, :]) % NLON) / NLON
    ang_s = 2.0 * np.pi * ((ns_[:, None] * m[None, :]) % NLON) / NLON
    return (s * np.cos(ang_c)).astype(np.float32), (-s * np.sin(ang_s)).astype(
        np.float32
    )


def fold_x(x):
    """x: (C, nlat, nlon) f32 -> xc (C, nlat, 361), xs (C, nlat, 359)."""
    xc = np.empty((x.shape[0], x.shape[1], NC_COS), dtype=np.float32)
    xc[..., 0] = x[..., 0]
    xc[..., NLON // 2] = x[..., NLON // 2]
    xc[..., 1 : NLON // 2] = x[..., 1 : NLON // 2] + x[..., : NLON // 2 : -1]
    xs = x[..., 1 : NLON // 2] - x[..., : NLON // 2 : -1]
    return xc, np.ascontiguousarray(xs.astype(np.float32))


def pack_stage_a_inputs(x, kperm):
    """x: (C, nlat, nlon) f32 -> xin (C//GA, 768, GA*362) bf16, mats (768, 362).
    Latitude columns are packed in kperm order (mirror-paired k-tiles)."""
    import ml_dtypes

    bf = ml_dtypes.bfloat16
    xc, xs = fold_x(x)  # (C, k, n')
    xc = xc[:, kperm]
    xs = xs[:, kperm]
    ng = x.shape[0] // GA
    xin = np.zeros((ng, 768, GA, MEVEN), dtype=bf)
    # [g, n', c, k] <- transpose of (g, c, k, n')
    xin[:, :NC_COS, :, :NLAT] = (
        xc.reshape(ng, GA, NLAT, NC_COS).transpose(0, 3, 1, 2).astype(bf)
    )
    xin[:, 384 : 384 + NC_SIN, :, :NLAT] = (
        xs.reshape(ng, GA, NLAT, NC_SIN).transpose(0, 3, 1, 2).astype(bf)
    )
    cosm, sinm = _dft_matrices()
    mats = np.zeros((768, MEVEN), dtype=bf)
    mats[:NC_COS, :MMAX] = cosm.astype(bf)
    mats[384 : 384 + NC_SIN, :MMAX] = sinm.astype(bf)
    return xin.reshape(ng, 768, GA * MEVEN), mats


def _windows(weights):
    """Per index-pair latitude windows: union of |W| support over the 8
    cores' m's, span forced uniform (in chunk count) within each pair."""
    wabs = np.abs(weights).max(axis=1)  # (m, k)
    thr = 1e-7 * wabs.max()
    win = []
    for i in range(MPC):
        ms = [NCORES * i + j for j in range(NCORES) if NCORES * i + j < MMAX]
        nz = np.nonzero(wabs[ms].max(axis=0) > thr)[0]
        klo, khi = (int(nz[0]), int(nz[-1]) + 1) if len(nz) else (0, NLAT)
        win.append([klo, khi])
    nkc_list, rc_list, klo_list = [], [], []
    for pi in range(0, MPC, 2):
        nkc = max(-(-(w[1] - w[0]) // 128) for w in win[pi : pi + 2])
        rc = max(-(-(w[1] - w[0]) // nkc) for w in win[pi : pi + 2])
        for i in (pi, pi + 1):
            nkc_list.append(nkc)
            rc_list.append(rc)  # chunk height (<=128), uniform within pair
            klo_list.append(win[i][0])
    return nkc_list, rc_list, klo_list


def m_list(j):
    return [NCORES * i + j for i in range(MPC) if NCORES * i + j < MMAX]


def _install_ntff_hook():
    """This image's antenv lacks axon_hooks; synthesize it so bass_utils'
    trace=True path can capture NTFFs via the axon PJRT .so."""
    import sys

    if "antenv.axon_hooks" in sys.modules:
        return
    import types

    mod = types.ModuleType("antenv.axon_hooks")
    state = {"hook": None}
    mod.set_axon_ntff_profile_hook = lambda h: state.__setitem__("hook", h)
    mod.get_axon_ntff_profile_hook = lambda: state["hook"]
    sys.modules["antenv.axon_hooks"] = mod
    try:
        import importlib.util as ilu

        spec = ilu.spec_from_file_location(
            "_trn_boot_hook", "/root/.axon_site/trn_agent_boot/trn_boot.py"
        )
        tb = ilu.module_from_spec(spec)
        spec.loader.exec_module(tb)
        mod.set_axon_ntff_profile_hook(
            tb._ntff_profile_via_ctypes("/opt/axon/libaxon_pjrt.so")
        )
    except Exception:
        pass


def _run(nc, in_maps, label):
    kw = {}
    if os.environ.get("SHT_TRACE"):
        import concourse.bass_utils as bu

        bu.upload_artifacts = lambda tmpdir: tmpdir  # no S3 in this sandbox
        _install_ntff_hook()
        kw = dict(trace=True)
    try:
        res = run_bass_kernel_spmd(nc, in_maps, core_ids=list(range(NCORES)), **kw)
    except Exception:
        if not kw:
            raise
        res = run_bass_kernel_spmd(nc, in_maps, core_ids=list(range(NCORES)))
    LAST_PERF[label] = res.exec_time_ns
    return res


def kernel(x, weights):
    import ml_dtypes

    bf = ml_dtypes.bfloat16
    x = np.asarray(x, dtype=np.float32).reshape(C, NLAT, NLON)
    weights = np.asarray(weights, dtype=np.float32)

    nkc_list, rc_list, klo_list = _windows(weights)
    # mirror-paired latitude tiles: per-k max m needed (from the windows),
    # then per-tile m-width = max over the tile's latitudes
    mneed = np.zeros(NLAT, dtype=np.int64)
    for i in range(MPC):
        klo = klo_list[i]
        khi = min(NLAT, klo + nkc_list[i] * rc_list[i])
        hi = min(MMAX - 1, 8 * i + 7)
        mneed[klo:khi] = np.maximum(mneed[klo:khi], hi + 1)
    kperm = np.concatenate(
        [np.arange(0, 64), np.arange(297, 361),
         np.arange(64, 128), np.arange(233, 297),
         np.arange(128, 233)]
    )
    mts = []
    for k0, kp in K_TILES:
        w = int(mneed[kperm[k0 : k0 + kp]].max())
        mts.append(min(MEVEN, w + (w % 2)))

    xin, mats = pack_stage_a_inputs(x, kperm)
    nc_a = build_stage_a(mts)
    in_maps = [
        {"xin": xin[j * NGA : (j + 1) * NGA], "mats": mats} for j in range(NCORES)
    ]
    res_a = _run(nc_a, in_maps, "stage_a")
    # reassemble XF[c, k, m] re/im from the three k-tile outputs
    xfr = np.zeros((C, NLAT, MMAX), dtype=xin.dtype)
    xfi = np.zeros((C, NLAT, MMAX), dtype=xin.dtype)
    for t, (k0, kp) in enumerate(K_TILES):
        mt = mts[t]
        piece = np.concatenate(
            [np.asarray(r[f"xf{t}"]) for r in res_a.results], axis=0
        ).reshape(NCORES * NGA, kp, GA, 2, mt)  # (gglob, kt-row, c, ri, m)
        piece = piece.transpose(0, 2, 3, 1, 4).reshape(C, 2, kp, mt)
        rows = kperm[k0 : k0 + kp]
        xfr[:, rows, : min(mt, MMAX)] = piece[:, 0, :, :MMAX]
        xfi[:, rows, : min(mt, MMAX)] = piece[:, 1, :, :MMAX]
    wtf = weights.transpose(0, 2, 1)  # (m, k, l) f32
    in_maps_b = []
    for j in range(NCORES):
        xwj = np.zeros((MPC // 2, 128, XW_W), dtype=bf)
        for pp in range(MPC // 2):
            pi = 2 * pp
            nkc, rc = nkc_list[pi], rc_list[pi]
            Lp = LMAX - 8 * pi
            cols = []
            for il in range(2):
                i = pi + il
                m = NCORES * i + j
                klo = klo_list[i]
                khi = min(NLAT, klo + nkc * rc)
                span = khi - klo
                pad = np.zeros((nkc * rc, NRIC), dtype=bf)
                if m < MMAX:
                    pad[:span, :C] = xfr[:, klo:khi, m].T
                    pad[:span, C:] = xfi[:, klo:khi, m].T
                cols.append(
                    pad.reshape(nkc, rc, NRIC).transpose(1, 0, 2).reshape(rc, nkc * NRIC)
                )
            for il in range(2):
                i = pi + il
                m = NCORES * i + j
                klo = klo_list[i]
                khi = min(NLAT, klo + nkc * rc)
                span = khi - klo
                wsrc = np.zeros((nkc * rc, Lp), dtype=bf)
                if m < MMAX:
                    wsrc[:span, : LMAX - 8 * i] = wtf[m, klo:khi, 8 * i :].astype(bf)
                cols.append(
                    wsrc.reshape(nkc, rc, Lp).transpose(1, 0, 2).reshape(rc, nkc * Lp)
                )
            blk = np.concatenate(cols, axis=1)
            xwj[pp, :rc, : blk.shape[1]] = blk
        in_maps_b.append({"xw": xwj})
    nc_b = build_stage_b(nkc_list, rc_list)
    res_b = _run(nc_b, in_maps_b, "stage_b")

    out = np.zeros((1, C, LMAX, MMAX), dtype=np.complex64)
    for j in range(NCORES):
        o = np.asarray(res_b.results[j]["outb"], dtype=np.float32)  # (23,128,3072)
        for pp in range(MPC // 2):
            pi = 2 * pp
            nfull = len(_ptiles(LMAX - 8 * pi)) - 1
            for il in range(2):
                i = pi + il
                m = NCORES * i + j
                if m >= MMAX:
                    continue
                Li = LMAX - 8 * i
                tiles = _ptiles(Li)
                parts = [
                    o[pp, :, (il * nfull + tl) * NRIC : (il * nfull + tl + 1) * NRIC]
                    for tl in range(nfull)
                ]
                lp_last = tiles[-1][1]
                parts.append(
                    o[pp, :lp_last, (2 * nfull + il) * NRIC : (2 * nfull + il + 1) * NRIC]
                )
                flat = np.concatenate(parts, axis=0)[:Li]  # (l - 8i, f)
                out[0, :, 8 * i :, m] = (flat[:, :C] + 1j * flat[:, C:]).T
    return out
